# revision 23
# baseline (speedup 1.0000x reference)
"""Windowed correlation kernel, v5 experiment:
  - matmul rhs streams directly from f2p via a 2-free-dim AP (no im2col)
  - 8 PSUM banks, bank-reuse waits batched per half-row (PE p-state ramp)
  - evacuations split 8 DVE / 8 ACT

Evac n (= r*16 + x0): x0 in 0..8 -> DVE (9/row), x0 in 9..15 -> ACT (7/row).
DVE ordinal r*9 + x0; ACT ordinal r*7 + (x0-9).
"""

import numpy as np

_B, _C, _H, _W = 8, 128, 128, 256
_K = 9
_ND = _K * _K
_BY, _BX = 8, 16
_NBY, _NBX = _H // _BY, _W // _BX
_NA, _NB = _BY + _K - 1, _BX + _K - 1
_NCOLS = _NA * _NB                       # 384
_HP, _WP = _H + _K - 1, _W + _K - 1
_NPS = 8
_ROW = 128 * _NBX * _NCOLS               # 786432
_NDVE = 8                                # DVE evacs per row (x0 0..7)
_NAE = _NBX - _NDVE                      # ACT evacs per row (x0 8..15)

_CACHE = {}


def _build_nc():
    from contextlib import ExitStack

    import concourse.bass as bass
    import concourse.mybir as mybir

    nc = bass.Bass()
    f1 = nc.dram_tensor(
        "f1", [_C, _NBY, _NBX * 128], mybir.dt.bfloat16, kind="ExternalInput"
    )
    f2 = nc.dram_tensor("f2", [_C, _HP, _WP], mybir.dt.bfloat16, kind="ExternalInput")
    out = nc.dram_tensor(
        "out", [_NBY, 128, _NBX * _NCOLS], mybir.dt.bfloat16, kind="ExternalOutput"
    )

    inv_c = 1.0 / _C
    rows = _NBY

    with ExitStack() as ctx:
        f1blk = ctx.enter_context(
            nc.sbuf_tensor([_C, _NBY * _NBX * 128], mybir.dt.bfloat16)
        )
        f2p = ctx.enter_context(nc.sbuf_tensor([_C, _HP * _WP], mybir.dt.bfloat16))
        stage = [
            ctx.enter_context(
                nc.sbuf_tensor(f"stg{i}", [_C, _NBX * _NCOLS], mybir.dt.bfloat16)
            )
            for i in range(3)
        ]
        psum = [
            ctx.enter_context(
                nc.psum_tensor(f"ps{i}", [128, _NCOLS], mybir.dt.float32)
            )
            for i in range(_NPS)
        ]
        s_f1 = ctx.enter_context(nc.semaphore(name="s_f1"))
        s_f2 = ctx.enter_context(nc.semaphore(name="s_f2"))
        s_pe = ctx.enter_context(nc.semaphore(name="s_pe"))     # +1 per matmul
        s_dve = ctx.enter_context(nc.semaphore(name="s_dve"))   # +1 per DVE evac
        s_ae = ctx.enter_context(nc.semaphore(name="s_ae"))     # +1 per ACT evac
        s_st = ctx.enter_context(nc.semaphore(name="s_st"))     # +16 per dump
        s_f2b = ctx.enter_context(nc.semaphore(name="s_f2b"))   # f2 chunk 1 (sync)

        blk = ctx.enter_context(nc.Block())

        def f2_chunk(k):
            src = bass.AP(
                tensor=f2,
                offset=k * _BY * _WP,
                ap=[[_HP * _WP, _C], [1, _BY * _WP]],
            )
            return f2p[:, k * _BY * _WP : (k + 1) * _BY * _WP], src

        def f1_chunk(k):
            src = bass.AP(
                tensor=f1,
                offset=k * _NBX * 128,
                ap=[[_NBY * _NBX * 128, _C], [1, _NBX * 128]],
            )
            return f1blk[:, k * _NBX * 128 : (k + 1) * _NBX * 128], src

        @blk.gpsimd
        def _(gpsimd):
            # chunk 1 is loaded by the idle sync queue at startup
            for o, k in enumerate([0] + list(range(2, _NBY + 1))):
                if o >= 1:
                    gpsimd.wait_ge(s_f2, o * 16)
                dst, src = f2_chunk(k)
                gpsimd.dma_start(dst, src).then_inc(s_f2, 16)

        @blk.sync
        def _(sync):
            dst0, src0 = f2_chunk(1)
            sync.dma_start(dst0, src0).then_inc(s_f2b, 16)
            for r in range(rows):
                sync.wait_ge(s_dve, (r + 1) * _NDVE)
                sync.wait_ge(s_ae, (r + 1) * _NAE)
                if r >= 1:
                    sync.wait_ge(s_st, r * 16)
                dst = bass.AP(
                    tensor=out,
                    offset=(r % _NBY) * _ROW,
                    ap=[[_NBX * _NCOLS, 128], [1, _NBX * _NCOLS]],
                )
                sync.dma_start(dst, stage[r % 3][:, :]).then_inc(s_st, 16)
            sync.wait_ge(s_st, rows * 16)

        @blk.scalar
        def _(scalar):
            dstf, srcf = f1_chunk(0)
            scalar.dma_start(dstf, srcf).then_inc(s_f1, 16)
            dstf, srcf = f1_chunk(1)
            scalar.dma_start(dstf, srcf).then_inc(s_f1, 16)
            for r in range(rows):
                # prefetch f1 chunk r+2 (ordering: chunks <= r+1 done)
                if r + 2 < rows:
                    scalar.wait_ge(s_f1, (r + 2) * 16)
                    dstf, srcf = f1_chunk(r + 2)
                    scalar.dma_start(dstf, srcf).then_inc(s_f1, 16)
                # ACT evacs: x0 9..15 of row r
                if r >= 3:
                    scalar.wait_ge(s_st, (r - 2) * 16)
                for x0 in range(_NDVE, _NBX):
                    n = r * _NBX + x0
                    scalar.wait_ge(s_pe, n + 1)
                    st = stage[r % 3][:, x0 * _NCOLS : (x0 + 1) * _NCOLS]
                    nc.scalar.activation(
                        st,
                        psum[n % _NPS][:, :],
                        mybir.ActivationFunctionType.Copy,
                        scale=inv_c,
                    ).then_inc(s_ae, 1)

        @blk.tensor
        def _(tensor):
            for r in range(rows):
                y0 = r % _NBY
                tensor.wait_ge(s_f1, max(r + 1, 2) * 16)
                if r == 0:
                    tensor.wait_ge(s_f2b, 16)
                tensor.wait_ge(s_f2, (r + 1) * 16)
                for x0 in range(_NBX):
                    n = r * _NBX + x0
                    # batched bank-reuse waits: before quarter-group
                    # [n, n+4) ensure all evacs m <= n-5 are complete
                    # (covers the group's banks, last used by [n-8, n-4))
                    if n >= _NPS and x0 % 4 == 0:
                        if x0 == 0:
                            tensor.wait_ge(s_dve, r * _NDVE)
                            tensor.wait_ge(s_ae, (r - 1) * _NAE + (12 - _NDVE))
                        elif x0 == 4:
                            tensor.wait_ge(s_ae, r * _NAE)
                        elif x0 == 8:
                            tensor.wait_ge(s_dve, r * _NDVE + 4)
                        else:
                            tensor.wait_ge(s_dve, r * _NDVE + 8)
                    rhs = bass.AP(
                        tensor=f2p,
                        offset=y0 * _BY * _WP + x0 * _BX,
                        ap=[[_HP * _WP, _C], [_WP, _NA], [1, _NB]],
                    )
                    lhsT = f1blk[:, (y0 * _NBX + x0) * 128 : (y0 * _NBX + x0 + 1) * 128]
                    nc.tensor.matmul(
                        psum[n % _NPS][:, :], lhsT, rhs, start=True, stop=True
                    ).then_inc(s_pe, 1)

        @blk.vector
        def _(vector):
            for r in range(rows):
                if r >= 3:
                    vector.wait_ge(s_st, (r - 2) * 16)
                for x0 in range(_NDVE):
                    n = r * _NBX + x0
                    vector.wait_ge(s_pe, n + 1)
                    st = stage[r % 3][:, x0 * _NCOLS : (x0 + 1) * _NCOLS]
                    nc.vector.tensor_scalar_mul(
                        st, psum[n % _NPS][:, :], inv_c
                    ).then_inc(s_dve, 1)

    return nc


def _pack_f1(f1_core: np.ndarray) -> np.ndarray:
    import ml_dtypes

    v = f1_core.reshape(_C, _NBY, _BY, _NBX, _BX)
    v = v.transpose(0, 1, 3, 4, 2)  # c, y0, x0, rx, ry
    return np.ascontiguousarray(v.reshape(_C, _NBY, _NBX * 128)).astype(
        ml_dtypes.bfloat16
    )


def _core_inputs(f1_core: np.ndarray, f2_core: np.ndarray) -> dict:
    import ml_dtypes

    f2p = np.zeros((_C, _HP, _WP), dtype=ml_dtypes.bfloat16)
    f2p[:, 4 : 4 + _H, 4 : 4 + _W] = f2_core.astype(ml_dtypes.bfloat16)
    return {"f1": _pack_f1(f1_core), "f2": f2p}


def _unshard_core(out_core: np.ndarray) -> np.ndarray:
    flat = np.ascontiguousarray(out_core).reshape(-1)
    assert flat.size == _NBY * _ROW
    sz = flat.itemsize
    view = np.lib.stride_tricks.as_strided(
        flat,
        shape=(_K, _K, _NBY, _BY, _NBX, _BX),
        strides=(
            _NB * sz,
            1 * sz,
            _ROW * sz,
            (6144 + _NB) * sz,
            _NCOLS * sz,
            (8 * 6144 + 1) * sz,
        ),
    )
    return view.astype(np.float32).reshape(_ND, _H, _W)


def kernel(feature1: np.ndarray, feature2: np.ndarray) -> np.ndarray:
    from concurrent.futures import ThreadPoolExecutor

    from concourse.bass_utils import run_bass_kernel_spmd

    if "nc" not in _CACHE:
        _CACHE["nc"] = _build_nc()
    nc = _CACHE["nc"]

    f1 = np.ascontiguousarray(np.asarray(feature1), dtype=np.float32)
    f2 = np.ascontiguousarray(np.asarray(feature2), dtype=np.float32)
    in_maps = [_core_inputs(f1[i], f2[i]) for i in range(_B)]
    res = run_bass_kernel_spmd(nc, in_maps, core_ids=list(range(_B)))
    outp = np.empty((_B, _ND, _H, _W), dtype=np.float32)

    def one(i):
        outp[i] = _unshard_core(res.results[i]["out"])

    with ThreadPoolExecutor(max_workers=_B) as ex:
        list(ex.map(one, range(_B)))
    return outp


# revision 24
# speedup vs baseline: 1.1017x; 1.1017x over previous
"""Windowed correlation kernel, v5 experiment:
  - matmul rhs streams directly from f2p via a 2-free-dim AP (no im2col)
  - 8 PSUM banks, bank-reuse waits batched per half-row (PE p-state ramp)
  - evacuations split 8 DVE / 8 ACT

Evac n (= r*16 + x0): x0 in 0..8 -> DVE (9/row), x0 in 9..15 -> ACT (7/row).
DVE ordinal r*9 + x0; ACT ordinal r*7 + (x0-9).
"""

import numpy as np

_B, _C, _H, _W = 8, 128, 128, 256
_K = 9
_ND = _K * _K
_BY, _BX = 8, 16
_NBY, _NBX = _H // _BY, _W // _BX
_NA, _NB = _BY + _K - 1, _BX + _K - 1
_NCOLS = _NA * _NB                       # 384
_HP, _WP = _H + _K - 1, _W + _K - 1
_NPS = 8
_ROW = 128 * _NBX * _NCOLS               # 786432
_NDVE = 9                                # DVE evacs per row (x0 0..8)
_NAE = _NBX - _NDVE                      # ACT evacs per row (x0 9..15)

_CACHE = {}


def _build_nc():
    from contextlib import ExitStack

    import concourse.bass as bass
    import concourse.mybir as mybir

    nc = bass.Bass()
    f1 = nc.dram_tensor(
        "f1", [_C, _NBY, _NBX * 128], mybir.dt.bfloat16, kind="ExternalInput"
    )
    f2 = nc.dram_tensor("f2", [_C, _HP, _WP], mybir.dt.bfloat16, kind="ExternalInput")
    out = nc.dram_tensor(
        "out", [_NBY, 128, _NBX * _NCOLS], mybir.dt.bfloat16, kind="ExternalOutput"
    )

    inv_c = 1.0 / _C
    rows = _NBY

    with ExitStack() as ctx:
        f1blk = ctx.enter_context(
            nc.sbuf_tensor([_C, _NBY * _NBX * 128], mybir.dt.bfloat16)
        )
        f2p = ctx.enter_context(nc.sbuf_tensor([_C, _HP * _WP], mybir.dt.bfloat16))
        stage = [
            ctx.enter_context(
                nc.sbuf_tensor(f"stg{i}", [_C, _NBX * _NCOLS], mybir.dt.bfloat16)
            )
            for i in range(3)
        ]
        psum = [
            ctx.enter_context(
                nc.psum_tensor(f"ps{i}", [128, _NCOLS], mybir.dt.float32)
            )
            for i in range(_NPS)
        ]
        s_f1 = ctx.enter_context(nc.semaphore(name="s_f1"))
        s_f2 = ctx.enter_context(nc.semaphore(name="s_f2"))
        s_pe = ctx.enter_context(nc.semaphore(name="s_pe"))     # +1 per matmul
        s_dve = ctx.enter_context(nc.semaphore(name="s_dve"))   # +1 per DVE evac
        s_ae = ctx.enter_context(nc.semaphore(name="s_ae"))     # +1 per ACT evac
        s_st = ctx.enter_context(nc.semaphore(name="s_st"))     # +16 per dump
        s_f2b = ctx.enter_context(nc.semaphore(name="s_f2b"))   # f2 chunk 1 (sync)

        blk = ctx.enter_context(nc.Block())

        def f2_chunk(k):
            src = bass.AP(
                tensor=f2,
                offset=k * _BY * _WP,
                ap=[[_HP * _WP, _C], [1, _BY * _WP]],
            )
            return f2p[:, k * _BY * _WP : (k + 1) * _BY * _WP], src

        def f1_chunk(k):
            src = bass.AP(
                tensor=f1,
                offset=k * _NBX * 128,
                ap=[[_NBY * _NBX * 128, _C], [1, _NBX * 128]],
            )
            return f1blk[:, k * _NBX * 128 : (k + 1) * _NBX * 128], src

        @blk.gpsimd
        def _(gpsimd):
            # chunk 1 is loaded by the idle sync queue at startup
            for o, k in enumerate([0] + list(range(2, _NBY + 1))):
                if o >= 1:
                    gpsimd.wait_ge(s_f2, o * 16)
                dst, src = f2_chunk(k)
                gpsimd.dma_start(dst, src).then_inc(s_f2, 16)

        @blk.sync
        def _(sync):
            dst0, src0 = f2_chunk(1)
            sync.dma_start(dst0, src0).then_inc(s_f2b, 16)
            for r in range(rows - 1):
                sync.wait_ge(s_dve, (r + 1) * _NDVE)
                sync.wait_ge(s_ae, (r + 1) * _NAE)
                if r >= 1:
                    sync.wait_ge(s_st, r * 16)
                dst = bass.AP(
                    tensor=out,
                    offset=(r % _NBY) * _ROW,
                    ap=[[_NBX * _NCOLS, 128], [1, _NBX * _NCOLS]],
                )
                sync.dma_start(dst, stage[r % 3][:, :]).then_inc(s_st, 16)
            # last row: 4 quarter-dumps pipelined with its evacuations
            r = rows - 1
            sync.wait_ge(s_st, r * 16)
            q_elems = 4 * _NCOLS
            for q in range(4):
                if q < 2:
                    sync.wait_ge(s_dve, r * _NDVE + 4 * (q + 1))
                elif q == 2:
                    sync.wait_ge(s_dve, (r + 1) * _NDVE)
                    sync.wait_ge(s_ae, r * _NAE + 3)
                else:
                    sync.wait_ge(s_ae, (r + 1) * _NAE)
                dst = bass.AP(
                    tensor=out,
                    offset=(r % _NBY) * _ROW + q * q_elems,
                    ap=[[_NBX * _NCOLS, 128], [1, q_elems]],
                )
                sync.dma_start(
                    dst, stage[r % 3][:, q * q_elems : (q + 1) * q_elems]
                ).then_inc(s_st, 16)
            sync.wait_ge(s_st, (rows + 3) * 16)

        @blk.scalar
        def _(scalar):
            dstf, srcf = f1_chunk(0)
            scalar.dma_start(dstf, srcf).then_inc(s_f1, 16)
            scalar.wait_ge(s_f1, 16)
            dstf, srcf = f1_chunk(1)
            scalar.dma_start(dstf, srcf).then_inc(s_f1, 16)
            for r in range(rows):
                # prefetch f1 chunk r+2 (ordering: chunks <= r+1 done)
                if r + 2 < rows:
                    scalar.wait_ge(s_f1, (r + 2) * 16)
                    dstf, srcf = f1_chunk(r + 2)
                    scalar.dma_start(dstf, srcf).then_inc(s_f1, 16)
                # ACT evacs: x0 9..15 of row r
                if r >= 3:
                    scalar.wait_ge(s_st, (r - 2) * 16)
                for x0 in range(_NDVE, _NBX):
                    n = r * _NBX + x0
                    scalar.wait_ge(s_pe, n + 1)
                    st = stage[r % 3][:, x0 * _NCOLS : (x0 + 1) * _NCOLS]
                    nc.scalar.activation(
                        st,
                        psum[n % _NPS][:, :],
                        mybir.ActivationFunctionType.Copy,
                        scale=inv_c,
                    ).then_inc(s_ae, 1)

        @blk.tensor
        def _(tensor):
            for r in range(rows):
                y0 = r % _NBY
                tensor.wait_ge(s_f1, (r + 1) * 16)
                if r == 0:
                    tensor.wait_ge(s_f2b, 16)
                tensor.wait_ge(s_f2, (r + 1) * 16)
                for x0 in range(_NBX):
                    n = r * _NBX + x0
                    # batched bank-reuse waits: before quarter-group
                    # [n, n+4) ensure all evacs m <= n-5 are complete
                    # (covers the group's banks, last used by [n-8, n-4))
                    if n >= _NPS and x0 % 4 == 0:
                        if x0 == 0:
                            tensor.wait_ge(s_dve, r * _NDVE)
                            tensor.wait_ge(s_ae, (r - 1) * _NAE + (12 - _NDVE))
                        elif x0 == 4:
                            tensor.wait_ge(s_ae, r * _NAE)
                        elif x0 == 8:
                            tensor.wait_ge(s_dve, r * _NDVE + 4)
                        else:
                            tensor.wait_ge(s_dve, r * _NDVE + 8)
                    rhs = bass.AP(
                        tensor=f2p,
                        offset=y0 * _BY * _WP + x0 * _BX,
                        ap=[[_HP * _WP, _C], [_WP, _NA], [1, _NB]],
                    )
                    lhsT = f1blk[:, (y0 * _NBX + x0) * 128 : (y0 * _NBX + x0 + 1) * 128]
                    nc.tensor.matmul(
                        psum[n % _NPS][:, :], lhsT, rhs, start=True, stop=True
                    ).then_inc(s_pe, 1)

        @blk.vector
        def _(vector):
            for r in range(rows):
                if r >= 3:
                    vector.wait_ge(s_st, (r - 2) * 16)
                for x0 in range(_NDVE):
                    n = r * _NBX + x0
                    vector.wait_ge(s_pe, n + 1)
                    st = stage[r % 3][:, x0 * _NCOLS : (x0 + 1) * _NCOLS]
                    nc.vector.tensor_scalar_mul(
                        st, psum[n % _NPS][:, :], inv_c
                    ).then_inc(s_dve, 1)

    return nc


def _pack_f1(f1_core: np.ndarray) -> np.ndarray:
    import ml_dtypes

    v = f1_core.reshape(_C, _NBY, _BY, _NBX, _BX)
    v = v.transpose(0, 1, 3, 4, 2)  # c, y0, x0, rx, ry
    return np.ascontiguousarray(v.reshape(_C, _NBY, _NBX * 128)).astype(
        ml_dtypes.bfloat16
    )


def _core_inputs(f1_core: np.ndarray, f2_core: np.ndarray) -> dict:
    import ml_dtypes

    f2p = np.zeros((_C, _HP, _WP), dtype=ml_dtypes.bfloat16)
    f2p[:, 4 : 4 + _H, 4 : 4 + _W] = f2_core.astype(ml_dtypes.bfloat16)
    return {"f1": _pack_f1(f1_core), "f2": f2p}


def _unshard_core(out_core: np.ndarray) -> np.ndarray:
    flat = np.ascontiguousarray(out_core).reshape(-1)
    assert flat.size == _NBY * _ROW
    sz = flat.itemsize
    view = np.lib.stride_tricks.as_strided(
        flat,
        shape=(_K, _K, _NBY, _BY, _NBX, _BX),
        strides=(
            _NB * sz,
            1 * sz,
            _ROW * sz,
            (6144 + _NB) * sz,
            _NCOLS * sz,
            (8 * 6144 + 1) * sz,
        ),
    )
    return view.astype(np.float32).reshape(_ND, _H, _W)


def kernel(feature1: np.ndarray, feature2: np.ndarray) -> np.ndarray:
    from concurrent.futures import ThreadPoolExecutor

    from concourse.bass_utils import run_bass_kernel_spmd

    if "nc" not in _CACHE:
        _CACHE["nc"] = _build_nc()
    nc = _CACHE["nc"]

    f1 = np.ascontiguousarray(np.asarray(feature1), dtype=np.float32)
    f2 = np.ascontiguousarray(np.asarray(feature2), dtype=np.float32)
    in_maps = [_core_inputs(f1[i], f2[i]) for i in range(_B)]
    res = run_bass_kernel_spmd(nc, in_maps, core_ids=list(range(_B)))
    outp = np.empty((_B, _ND, _H, _W), dtype=np.float32)

    def one(i):
        outp[i] = _unshard_core(res.results[i]["out"])

    with ThreadPoolExecutor(max_workers=_B) as ex:
        list(ex.map(one, range(_B)))
    return outp
